# revision 4
# baseline (speedup 1.0000x reference)
"""Trainium2 Bass kernel for the histogram_binning problem.

Math (per batch sample b):
  h = x[b] viewed as [C, N]  (C=2208 channels, N=196 positions)
  z[n, k] = sum_c h[c, n] * W[k, c] + bias[k]          (K=200 classes)
  max_val[n]  = max_k softmax(z[n,:]) = 1 / sum_k exp(z[n,k] - zmax[n])
  max_ids[n]  = argmax_k z[n, :]
  norm = max_val / ||max_val||_2
  p_r[k] = (sum_{n: ids[n]=k} max_val[n]) / L1   (L2 scale cancels under L1 norm)
  out[c, n] = x[c, n] * (1 + norm[n])

Distribution: pure data parallel, batch 64 -> 8 cores x 8 samples.

Implementation notes:
 - x is host-padded [C=2208] -> [CP=2304 = 18*128] rows; row 2208 is all ones
   and W row 2208 is fc_b, folding the bias add into the contraction. The
   remaining pad rows are zeros on both sides.
 - argmax one-hot is computed as (z - zmax == 0) elementwise on the PSUM tile;
   the scatter-add histogram is a [n,1]^T x [n,200] matmul.
"""

import numpy as np

import concourse.bass as bass
import concourse.bacc as bacc
import concourse.mybir as mybir
import concourse.tile as tile
from concourse.bass_utils import run_bass_kernel_spmd
from concourse.masks import make_identity

F32 = mybir.dt.float32

B = 64
C = 2208
CP = 2304            # padded channel dim: 18 * 128 (row 2208 = ones for bias)
H = W = 14
N = H * W            # 196
K = 200
NCORES = 8
BPC = B // NCORES    # 8 samples per core
CT = CP // 128       # 18 contraction tiles of 128
NT = ((0, 128), (128, 68))   # (offset, size) tiles of N=196


def _build_nc() -> bass.Bass:
    nc = bacc.Bacc(None, target_bir_lowering=False, debug=False)
    x_d = nc.declare_dram_parameter("xs", [BPC, CP, N], F32, isOutput=False)
    wp_d = nc.declare_dram_parameter("wp", [CT, 128, K], F32, isOutput=False)
    out_d = nc.declare_dram_parameter("yo", [BPC, C, N], F32, isOutput=True)
    pr_d = nc.declare_dram_parameter("pr", [BPC, K], F32, isOutput=True)

    with tile.TileContext(nc) as tc:
        with (
            tc.tile_pool(name="consts", bufs=1) as consts,
            tc.tile_pool(name="xpool", bufs=1) as xpool,
            tc.tile_pool(name="maskp", bufs=4) as maskp,
            tc.tile_pool(name="escr", bufs=3) as escr,
            tc.tile_pool(name="stats", bufs=6) as stats,
            tc.tile_pool(name="mvp", bufs=6) as mvp,
            tc.tile_pool(name="brow", bufs=3) as brow,
            tc.tile_pool(name="tinyp", bufs=3) as tinyp,
            tc.tile_pool(name="bcsb", bufs=3) as bcsb,
            tc.tile_pool(name="psz", bufs=3, space="PSUM") as psz_pool,
            tc.tile_pool(name="pspr", bufs=2, space="PSUM") as pspr_pool,
            tc.tile_pool(name="pstr", bufs=1, space="PSUM") as pstr_pool,
            tc.tile_pool(name="psbc", bufs=2, space="PSUM") as psbc_pool,
        ):
            # --- constants ---
            w_sb = consts.tile([128, CT, K], F32)
            nc.sync.dma_start(out=w_sb, in_=wp_d.rearrange("t p k -> p t k"))
            ident = consts.tile([128, 128], F32)
            make_identity(nc, ident)
            ones_row = consts.tile([1, 128], F32)
            nc.gpsimd.memset(ones_row, 1.0)

            for b in range(BPC):
                # --- load x[b] as [CP, N] = [128, 18, N] in one DMA ---
                x_b = xpool.tile([128, CT, N], F32, tag=f"x{b}")
                nc.sync.dma_start(
                    out=x_b,
                    in_=x_d[b].rearrange("(t p) n -> p t n", p=128),
                )

                pspr = pspr_pool.tile([1, K], F32, tag="pr")
                pstr = pstr_pool.tile([1, N], F32, tag="tr")

                for i, (noff, nsz) in enumerate(NT):
                    # z = x^T W  (+bias via ones row), accumulated over 18 chunks
                    psz = psz_pool.tile([128, K], F32, tag="z")
                    for t in range(CT):
                        nc.tensor.matmul(
                            psz[:nsz, :],
                            lhsT=x_b[:, t, noff : noff + nsz],
                            rhs=w_sb[:, t, :],
                            start=(t == 0),
                            stop=(t == CT - 1),
                        )
                    # negmax[n] = -max_k z
                    negmax = stats.tile([128, 1], F32, tag="negmax")
                    nc.vector.tensor_reduce(
                        out=negmax[:nsz],
                        in_=psz[:nsz, :],
                        axis=mybir.AxisListType.X,
                        op=mybir.AluOpType.max,
                        negate=True,
                    )
                    # sumexp[n] = sum_k exp(z - zmax)
                    e_scr = escr.tile([128, K], F32, tag="escr")
                    sumexp = stats.tile([128, 1], F32, tag="sumexp")
                    nc.scalar.activation(
                        out=e_scr[:nsz],
                        in_=psz[:nsz, :],
                        func=mybir.ActivationFunctionType.Exp,
                        bias=negmax[:nsz],
                        scale=1.0,
                        accum_out=sumexp[:nsz],
                    )
                    # one-hot argmax mask: (z + negmax) == 0
                    mask = maskp.tile([128, K], F32, tag="mask")
                    nc.vector.tensor_scalar(
                        mask[:nsz, :],
                        psz[:nsz, :],
                        negmax[:nsz],
                        0.0,
                        op0=mybir.AluOpType.add,
                        op1=mybir.AluOpType.is_equal,
                    )
                    # max_val[n] = 1 / sumexp
                    mv = mvp.tile([128, 1], F32, tag=f"mv{i}")
                    nc.vector.reciprocal(mv[:nsz], sumexp[:nsz])
                    # histogram scatter: p_r_raw[1, k] += mv^T @ mask
                    nc.tensor.matmul(
                        pspr[:, :],
                        lhsT=mv[:nsz, :],
                        rhs=mask[:nsz, :],
                        start=(i == 0),
                        stop=(i == 1),
                    )
                    # transpose max_val column into a row [1, N]
                    nc.tensor.transpose(
                        pstr[0:1, noff : noff + nsz],
                        mv[:nsz, :],
                        ident[:nsz, :nsz],
                    )

                # --- per-sample tail: normalize + broadcast + scale ---
                mvrow = brow.tile([1, N], F32, tag="mvrow")
                nc.scalar.copy(mvrow, pstr[0:1, :])
                sq_scr = brow.tile([1, N], F32, tag="sqscr")
                ssq = tinyp.tile([1, 1], F32, tag="ssq")
                nc.scalar.activation(
                    out=sq_scr,
                    in_=mvrow,
                    func=mybir.ActivationFunctionType.Square,
                    accum_out=ssq,
                )
                l2 = tinyp.tile([1, 1], F32, tag="l2")
                nc.scalar.activation(
                    out=l2, in_=ssq, func=mybir.ActivationFunctionType.Sqrt
                )
                rl2 = tinyp.tile([1, 1], F32, tag="rl2")
                nc.vector.reciprocal(rl2, l2)
                # n1row = max_val_row * rl2 + 1  == 1 + norm
                n1row = brow.tile([1, N], F32, tag="n1row")
                nc.vector.tensor_scalar(
                    n1row,
                    mvrow,
                    rl2,
                    1.0,
                    op0=mybir.AluOpType.mult,
                    op1=mybir.AluOpType.add,
                )
                # broadcast row to all 128 partitions: ones[1,128]^T @ n1row[1,N]
                psbc = psbc_pool.tile([128, N], F32, tag="bc")
                nc.tensor.matmul(psbc, lhsT=ones_row, rhs=n1row)
                bc_sb = bcsb.tile([128, N], F32, tag="bcsb")
                nc.scalar.copy(bc_sb, psbc)

                # out = x * (1 + norm), in place over x_b
                nc.vector.tensor_tensor(
                    x_b[:, : CT - 1, :],
                    x_b[:, : CT - 1, :],
                    bc_sb[:, None, :].to_broadcast((128, CT - 1, N)),
                    op=mybir.AluOpType.mult,
                )
                nc.vector.tensor_tensor(
                    x_b[:32, CT - 1, :],
                    x_b[:32, CT - 1, :],
                    bc_sb[:32, :],
                    op=mybir.AluOpType.mult,
                )
                nc.sync.dma_start(
                    out=out_d[b, : 128 * (CT - 1), :].rearrange(
                        "(t p) n -> p t n", p=128
                    ),
                    in_=x_b[:, : CT - 1, :],
                )
                nc.sync.dma_start(
                    out=out_d[b, 128 * (CT - 1) :, :],
                    in_=x_b[:32, CT - 1, :],
                )

                # --- p_r row: L1 normalize and store ---
                l1 = tinyp.tile([1, 1], F32, tag="l1")
                nc.vector.tensor_reduce(
                    out=l1,
                    in_=pspr[0:1, :],
                    axis=mybir.AxisListType.X,
                    op=mybir.AluOpType.add,
                )
                rl1 = tinyp.tile([1, 1], F32, tag="rl1")
                nc.vector.reciprocal(rl1, l1)
                pr_sb = brow.tile([1, K], F32, tag="prsb")
                nc.vector.tensor_scalar_mul(pr_sb, pspr[0:1, :], rl1)
                nc.sync.dma_start(out=pr_d[b : b + 1, :], in_=pr_sb)

    nc.compile()
    return nc


_NC = None


def _get_nc():
    global _NC
    if _NC is None:
        _NC = _build_nc()
    return _NC


def _pack_weights(fc_w: np.ndarray, fc_b: np.ndarray) -> np.ndarray:
    """[CT, 128, K]: W^T padded to CP rows; row 2208 = fc_b, rest zeros."""
    wp = np.zeros((CP, K), dtype=np.float32)
    wp[:C] = fc_w.astype(np.float32, copy=False).T
    wp[C] = fc_b.astype(np.float32, copy=False)
    return np.ascontiguousarray(wp.reshape(CT, 128, K))


def _pad_x(x: np.ndarray) -> np.ndarray:
    """[B, CP, N]: x rows, then a ones row at 2208, zeros to 2304."""
    xp = np.zeros((B, CP, N), dtype=np.float32)
    xp[:, :C] = x.reshape(B, C, N)
    xp[:, C] = 1.0
    return xp


def _run(x, fc_w, fc_b, flag, trace=False, trace_cores=None):
    x = np.asarray(x, dtype=np.float32)
    xp = _pad_x(x)
    wp = _pack_weights(np.asarray(fc_w), np.asarray(fc_b))
    in_maps = [
        {"xs": np.ascontiguousarray(xp[i * BPC : (i + 1) * BPC]), "wp": wp}
        for i in range(NCORES)
    ]
    nc = _get_nc()
    res = run_bass_kernel_spmd(
        nc,
        in_maps,
        core_ids=list(range(NCORES)),
        trace=trace,
        **({"trace_cores": trace_cores} if trace_cores else {}),
    )
    out = np.concatenate([r["yo"] for r in res.results], axis=0)
    out = out.reshape(B, C, H, W)
    p_r = np.concatenate([r["pr"] for r in res.results], axis=0)
    if not int(np.asarray(flag)):
        p_r = np.zeros_like(p_r)
    return (out, p_r), res


def kernel(x, fc_w, fc_b, flag):
    (out, p_r), _ = _run(x, fc_w, fc_b, flag)
    return out, p_r
